# revision 1
# baseline (speedup 1.0000x reference)
# CapsuleNetwork Trainium2 kernel (8-core data parallel, 4 images/core).
#
# Per core:
#   conv1 3->256 k9 s1 (im2col K=243, bf16 matmuls) + relu
#   conv2 256->256 k9 s2 (81-tap PSUM accumulation, bf16, weights resident)
#   capsule squash (pixel-major), dynamic routing (3 iters) without
#   materializing u_hat:
#     F'[(r,i),(r',c)] = sum_p capsW[p,(r,i)] * exp(b)[p,(r',c)]; G = diag blocks
#     s[c,o]  = sum_{r,i} G[r,c,i] * route_w[r,c,i,o]   (per-class matmuls)
#     b     += caps @ T_block,  T_block[(r,i),(r,c)] = sum_o w[r,c,i,o] v[c,o]
import functools
from contextlib import ExitStack

import numpy as np
import ml_dtypes

import concourse.bass as bass
import concourse.tile as tile
from concourse import bacc
from concourse import mybir
from concourse.bass_utils import run_bass_kernel_spmd

BF = mybir.dt.bfloat16
F32 = mybir.dt.float32
AF = mybir.ActivationFunctionType
AX = mybir.AxisListType

NCORES = 8
B = 4              # images per core
K1 = 243           # 3*9*9 im2col contraction
NPIX1 = 3136       # 56*56 conv1 output pixels
N1CH = 448         # conv1 moving chunk (3136 = 7*448)
PIX = 576          # 24*24 conv2 output pixels
PIX_CHUNKS = [(0, 128), (128, 128), (256, 128), (384, 128), (512, 64)]
R, D, C, O = 32, 8, 10, 16


def _build_nc():
    nc = bacc.Bacc("TRN2", target_bir_lowering=False, debug=False)
    # register the squash-eps constant for activation bias use
    eps_t = nc.alloc_sbuf_tensor("const-eps", [128, 1], F32)
    nc.gpsimd.memset(eps_t.ap(), 1e-8)
    nc.const_aps.aps[(F32, 1e-8)] = eps_t.ap()
    nc.all_engine_barrier()
    x_d = nc.declare_dram_parameter("x", [B, K1, NPIX1], BF, isOutput=False)
    w1_d = nc.declare_dram_parameter("w1", [256, 256], BF, isOutput=False)
    b1_d = nc.declare_dram_parameter("b1", [256, 1], F32, isOutput=False)
    w2_d = nc.declare_dram_parameter("w2", [2, 2, 128, 81 * 128], BF, isOutput=False)
    b2_d = nc.declare_dram_parameter("b2", [256, 1], F32, isOutput=False)
    ws_d = nc.declare_dram_parameter("ws", [256, C * O], BF, isOutput=False)
    wcob_d = nc.declare_dram_parameter("wcob", [O, C, 256], BF, isOutput=False)
    maskg_d = nc.declare_dram_parameter("maskg", [2, 128, R * C], F32, isOutput=False)
    idf_d = nc.declare_dram_parameter("idf", [128, 128], F32, isOutput=False)
    idb_d = nc.declare_dram_parameter("idb", [128, 128], BF, isOutput=False)
    vout_d = nc.declare_dram_parameter("v_out", [B * C, O], F32, isOutput=True)

    with tile.TileContext(nc) as tc, ExitStack() as ctx:
        consts = ctx.enter_context(tc.tile_pool(name="consts", bufs=1))
        w1a = consts.tile([128, 256], BF, tag="w1a", name="w1a")
        w1b = consts.tile([115, 256], BF, tag="w1b", name="w1b")
        nc.gpsimd.dma_start(w1a, w1_d[0:128, :])
        nc.gpsimd.dma_start(w1b, w1_d[128:243, :])
        b1t = [consts.tile([128, 1], F32, tag=f"b1_{m}", name=f"b1_{m}") for m in range(2)]
        b2t = [consts.tile([128, 1], F32, tag=f"b2_{m}", name=f"b2_{m}") for m in range(2)]
        for m in range(2):
            nc.gpsimd.dma_start(b1t[m], b1_d[m * 128:(m + 1) * 128, :])
            nc.gpsimd.dma_start(b2t[m], b2_d[m * 128:(m + 1) * 128, :])
        ws_t = [consts.tile([128, C * O], BF, tag=f"ws{m}", name=f"ws{m}") for m in range(2)]
        for m in range(2):
            nc.gpsimd.dma_start(ws_t[m], ws_d[m * 128:(m + 1) * 128, :])
        wcob = consts.tile([O, C, 256], BF, tag="wcob", name="wcob")
        nc.gpsimd.dma_start(wcob, wcob_d[:, :, :])
        idf = consts.tile([128, 128], F32, tag="idf", name="idf")
        idb = consts.tile([128, 128], BF, tag="idb", name="idb")
        nc.gpsimd.dma_start(idf, idf_d[:, :])
        nc.gpsimd.dma_start(idb, idb_d[:, :])
        # block-diag masks: maskg[m][j, r*C+c] = (r == m*16 + j//8)
        maskg = [consts.tile([128, R * C], F32, tag=f"mg{m}", name=f"mg{m}")
                 for m in range(2)]
        for m in range(2):
            nc.gpsimd.dma_start(maskg[m], maskg_d[m])
        ones16 = consts.tile([16, 1], F32, tag="ones16", name="ones16")
        ones1 = consts.tile([1, 16], F32, tag="ones1", name="ones1")
        nc.vector.memset(ones16, 1.0)
        nc.vector.memset(ones1, 1.0)

        # ---- persistent caps tensors (written during conv phase) ----
        persist = ctx.enter_context(tc.tile_pool(name="persist", bufs=1))
        caps_bf = [persist.tile([128, B, 256], BF, tag=f"cbf{k}", name=f"cbf{k}")
                   for k in range(5)]                              # pixel-major squashed
        capsT = [[persist.tile([128, PIX], BF, tag=f"cT{b}_{g}", name=f"cT{b}_{g}")
                  for g in range(2)] for b in range(B)]            # channel-major squashed
        capsum = [persist.tile([128, B], F32, tag=f"cs{g}", name=f"cs{g}")
                  for g in range(2)]

        # ================= conv1 + conv2 + squash =================
        with tc.tile_pool(name="w2pool", bufs=1) as w2pool, \
             tc.tile_pool(name="h1pool", bufs=1) as h1pool:
            w2t = [[w2pool.tile([128, 81, 128], BF, tag=f"w2_{ig}_{og}", name=f"w2_{ig}_{og}")
                    for og in range(2)] for ig in range(2)]
            for og in range(2):  # all w2 via the SWDGE queue (off im2col's path)
                for ig in range(2):
                    nc.gpsimd.dma_start(w2t[ig][og].rearrange("p t m -> p (t m)"),
                                        w2_d[ig, og])
            # h1 phase-split along x so conv2 rhs reads are stride-1:
            # [128, b, y(56), phase(2), x'(28)]
            h1 = [h1pool.tile([128, B, 56, 2, 28], BF, tag=f"h1_{g}", name=f"h1_{g}")
                  for g in range(2)]

            # ---- conv1 ----
            with tc.tile_pool(name="imcol", bufs=2) as impool, \
                 tc.tile_pool(name="c1psum", bufs=6, space="PSUM") as c1psum:
                for b in range(B):
                    imA = impool.tile([128, NPIX1], BF, tag="imA", name="imA")
                    imB = impool.tile([115, NPIX1], BF, tag="imB", name="imB")
                    # split each image across both HWDGE queues (~100GB/s each)
                    nc.sync.dma_start(imA[0:64], x_d[b, 0:64, :])
                    nc.scalar.dma_start(imA[64:128], x_d[b, 64:128, :])
                    nc.sync.dma_start(imB[0:58], x_d[b, 128:186, :])
                    nc.scalar.dma_start(imB[58:115], x_d[b, 186:K1, :])
                    for m in range(2):
                        for n in range(7):  # 448 pixels = 8 rows of 56
                            ps = c1psum.tile([128, 8, 56], F32, tag="c1ps", name="c1ps")
                            nc.tensor.matmul(ps, w1a[:, m * 128:(m + 1) * 128],
                                             imA[:, n * N1CH:(n + 1) * N1CH],
                                             start=True, stop=False)
                            nc.tensor.matmul(ps, w1b[:, m * 128:(m + 1) * 128],
                                             imB[:, n * N1CH:(n + 1) * N1CH],
                                             start=False, stop=True)
                            for px in range(2):
                                eng = nc.scalar if px == 0 else nc.vector
                                if px == 0:
                                    nc.scalar.activation(
                                        h1[m][:, b, 8 * n:8 * n + 8, px, :],
                                        ps[:, :, px::2],
                                        AF.Relu, bias=b1t[m], scale=1.0)
                                else:
                                    # relu(x + bias) on DVE to split drain load
                                    nc.vector.tensor_scalar(
                                        h1[m][:, b, 8 * n:8 * n + 8, px, :],
                                        ps[:, :, px::2], b1t[m], 0.0,
                                        op0=mybir.AluOpType.add,
                                        op1=mybir.AluOpType.max)

            # ---- conv2 + squash (b-outer so squash overlaps next image) ----
            with tc.tile_pool(name="craw", bufs=1) as crawpool, \
                 tc.tile_pool(name="c2psum", bufs=4, space="PSUM") as c2psum, \
                 tc.tile_pool(name="tpsum", bufs=2, space="PSUM") as tpsum, \
                 tc.tile_pool(name="tbpsum", bufs=2, space="PSUM") as tbpsum, \
                 tc.tile_pool(name="pmraw", bufs=2) as pmpool, \
                 tc.tile_pool(name="sqtmp", bufs=4) as sqpool:
                capsT_raw = [[crawpool.tile([128, PIX], F32, tag=f"cr{b}_{g}", name=f"cr{b}_{g}")
                              for g in range(2)] for b in range(B)]
                for b in range(B):
                    for og in range(2):
                        pss = [c2psum.tile([128, 288], F32, tag="c2ps", name="c2ps")
                               for _ in range(2)]
                        for ig in range(2):
                            for t81 in range(81):
                                kh, kw = t81 // 9, t81 % 9
                                lhsT = w2t[ig][og][:, t81, :]
                                for y in range(2):
                                    rhs = h1[ig][:, b,
                                                 kh + 24 * y:kh + 24 * y + 24:2,
                                                 kw % 2, kw // 2:kw // 2 + 24]
                                    nc.tensor.matmul(
                                        pss[y], lhsT, rhs,
                                        start=(ig == 0 and t81 == 0),
                                        stop=(ig == 1 and t81 == 80))
                        for y in range(2):
                            nc.scalar.activation(
                                capsT_raw[b][og][:, y * 288:(y + 1) * 288], pss[y],
                                AF.Identity, bias=b2t[og], scale=1.0)
                    # pixel-major transpose + squash + transpose back
                    for k, (p0, ln) in enumerate(PIX_CHUNKS):
                        pm = pmpool.tile([128, 256], F32, tag="pm", name="pm")
                        for og in range(2):
                            tp = tpsum.tile([128, 128], F32, tag="tp", name="tp")
                            nc.tensor.transpose(tp[:ln, :],
                                                capsT_raw[b][og][:, p0:p0 + ln], idf)
                            nc.vector.tensor_copy(
                                pm[:ln, og * 128:(og + 1) * 128], tp[:ln, :])
                        pm3 = pm.rearrange("p (r i) -> p r i", i=D)
                        sq = sqpool.tile([128, R, D], F32, tag="sq", name="sq")
                        nc.scalar.activation(sq[:ln], pm3[:ln], AF.Square)
                        nsq = sqpool.tile([128, R], F32, tag="nsq", name="nsq")
                        nc.vector.reduce_sum(nsq[:ln], sq[:ln], axis=AX.X)
                        a = sqpool.tile([128, R], F32, tag="sqa", name="sqa")
                        nc.scalar.activation(a[:ln], nsq[:ln], AF.Sqrt, bias=1e-8)
                        nc.vector.scalar_tensor_tensor(
                            a[:ln], nsq[:ln], 1.0, a[:ln],
                            op0=mybir.AluOpType.add, op1=mybir.AluOpType.mult)
                        nc.vector.reciprocal(a[:ln], a[:ln])
                        nc.vector.tensor_mul(a[:ln], nsq[:ln], a[:ln])
                        cbf3 = caps_bf[k][:, b].rearrange("p (r i) -> p r i", i=D)
                        nc.vector.tensor_mul(
                            cbf3[:ln], pm3[:ln],
                            a[:ln].unsqueeze(2).broadcast_to([ln, R, D]))
                        for og in range(2):
                            tb = tbpsum.tile([128, 128], BF, tag="tb", name="tb")
                            nc.tensor.transpose(
                                tb[:, :ln],
                                caps_bf[k][:ln, b, og * 128:(og + 1) * 128],
                                idb[:ln, :ln])
                            nc.vector.tensor_copy(capsT[b][og][:, p0:p0 + ln],
                                                  tb[:, :ln])
                    for g in range(2):  # iter-0 capsule sums, off critical path
                        nc.vector.reduce_sum(capsum[g][:, b:b + 1], capsT[b][g],
                                             axis=AX.X)

        # ================= routing =================
        rpool = ctx.enter_context(tc.tile_pool(name="rpool", bufs=1))
        dpool = ctx.enter_context(tc.tile_pool(name="dtmp", bufs=4))
        blog = [rpool.tile([128, B, R, C], F32, tag=f"bl{k}", name=f"bl{k}")
                for k in range(5)]

        def v_squash(s4T, last):
            """s4T: psum [16 (o), B, C] -> v4T bf16 [16, B, C] (or writes v_out)."""
            with tc.tile_pool(name="vsq", bufs=1, space="PSUM") as vps:
                s4T_sb = dpool.tile([16, B, C], F32, tag="s4Tsb", name="s4Tsb")
                nc.vector.tensor_copy(s4T_sb, s4T)
                if last:
                    s4 = vps.tile([40, 16], F32, tag="s4", name="s4")
                    nc.tensor.transpose(s4, s4T_sb, idf[:16, :16])
                    sq = dpool.tile([40, 16], F32, tag="vsq", name="vsq")
                    nc.scalar.activation(sq, s4, AF.Square)
                    nsq = dpool.tile([40, 1], F32, tag="vnsq", name="vnsq")
                    nc.vector.reduce_sum(nsq, sq, axis=AX.X)
                    a = dpool.tile([40, 1], F32, tag="va", name="va")
                    nc.scalar.activation(a, nsq, AF.Sqrt, bias=1e-8)
                    nc.vector.scalar_tensor_tensor(
                        a, nsq, 1.0, a,
                        op0=mybir.AluOpType.add, op1=mybir.AluOpType.mult)
                    nc.vector.reciprocal(a, a)
                    nc.vector.tensor_mul(a, nsq, a)
                    vout = rpool.tile([40, 16], F32, tag="vout", name="vout")
                    nc.vector.tensor_mul(vout, s4, a.broadcast_to([40, 16]))
                    nc.sync.dma_start(vout_d[:, :], vout)
                    return None
                # row-major squash: partition-reduce |s|^2 via ones-matmul,
                # broadcast the scale back via a K=1 outer-product matmul.
                sqT = dpool.tile([16, B * C], F32, tag="vsqT", name="vsqT")
                nc.scalar.activation(sqT, s4T.rearrange("p b c -> p (b c)"),
                                     AF.Square)
                nsqr = vps.tile([1, 40], F32, tag="nsqr", name="nsqr")
                nc.tensor.matmul(nsqr, ones16, sqT, start=True, stop=True)
                a = dpool.tile([1, 40], F32, tag="var", name="var")
                nc.scalar.activation(a, nsqr, AF.Sqrt, bias=1e-8)
                nc.vector.scalar_tensor_tensor(
                    a, nsqr, 1.0, a,
                    op0=mybir.AluOpType.add, op1=mybir.AluOpType.mult)
                nc.vector.reciprocal(a, a)
                sgr = dpool.tile([1, 40], F32, tag="sgr", name="sgr")
                nc.vector.tensor_mul(sgr, nsqr, a)
                sgT = vps.tile([16, 40], F32, tag="sgT", name="sgT")
                nc.tensor.matmul(sgT, ones1, sgr, start=True, stop=True)
                v4T = rpool.tile([16, B, C], BF, tag="v4T", name="v4T")
                nc.vector.tensor_mul(v4T, s4T_sb,
                                     sgT.rearrange("p (b c) -> p b c", c=C))
                return v4T

        def b_update(v4T, it):
            """b_log += caps . T_block  (T = route_w . v, block-diag in r)."""
            T4 = [rpool.tile([128, B, R * C], BF, tag=f"T4_{m}", name=f"T4_{m}") for m in range(2)]
            with tc.tile_pool(name="t4ps", bufs=2, space="PSUM") as t4ps, \
                 tc.tile_pool(name="dps", bufs=4, space="PSUM") as dps:
                for m in range(2):
                    t4 = t4ps.tile([128, C, B], F32, tag="t4", name="t4")
                    for c in range(C):
                        nc.tensor.matmul(t4[:, c, :],
                                         wcob[:, c, m * 128:(m + 1) * 128],
                                         v4T[:, :, c], start=True, stop=True)
                    data = (t4.transpose([0, 2, 1]).unsqueeze(2)
                            .broadcast_to([128, B, R, C]))
                    mk = (maskg[m].rearrange("p (r c) -> p r c", c=C)
                          .unsqueeze(1).broadcast_to([128, B, R, C]))
                    nc.vector.tensor_mul(
                        T4[m].rearrange("p b (r c) -> p b r c", c=C), data, mk)
                for k, (p0, ln) in enumerate(PIX_CHUNKS):
                    for b in range(B):
                        dl = dps.tile([128, R, C], F32, tag="dl", name="dl")
                        for kc in range(2):
                            nc.tensor.matmul(dl[:ln], capsT[b][kc][:, p0:p0 + ln],
                                             T4[kc][:, b, :],
                                             start=(kc == 0), stop=(kc == 1))
                        if it == 0:
                            nc.vector.tensor_copy(blog[k][:ln, b], dl[:ln])
                        else:
                            nc.vector.tensor_add(blog[k][:ln, b], blog[k][:ln, b],
                                                 dl[:ln])

        def softmax_G():
            """softmax over c folded into caps; G = diag blocks of cw.T @ e."""
            e = []
            cw = []
            for k, (p0, ln) in enumerate(PIX_CHUNKS):
                et = rpool.tile([128, B, R, C], BF, tag=f"e{k}", name=f"e{k}")
                nc.scalar.activation(et[:ln], blog[k][:ln], AF.Exp)
                den = dpool.tile([128, B, R], F32, tag="den", name="den")
                nc.vector.reduce_sum(den[:ln], et[:ln], axis=AX.X)
                nc.vector.reciprocal(den[:ln], den[:ln])
                cwt = rpool.tile([128, B, R, D], BF, tag=f"cw{k}", name=f"cw{k}")
                cbf4 = caps_bf[k].rearrange("p b (r i) -> p b r i", i=D)
                nc.vector.tensor_mul(
                    cwt[:ln], cbf4[:ln],
                    den[:ln].unsqueeze(3).broadcast_to([ln, B, R, D]))
                e.append(et)
                cw.append(cwt)
            Gp = [rpool.tile([128, B, C], BF, tag=f"G{m}", name=f"G{m}") for m in range(2)]
            for m in range(2):
                with tc.tile_pool(name="fps", bufs=1, space="PSUM") as fps:
                    F4 = fps.tile([128, B, 512], F32, tag="F4", name="F4")
                    for k, (p0, ln) in enumerate(PIX_CHUNKS):
                        for b in range(B):
                            cwf = cw[k][:, b].rearrange("p r i -> p (r i)")
                            ef = e[k][:, b].rearrange("p r c -> p (r c)")
                            nc.tensor.matmul(F4[:, b, :R * C],
                                             cwf[:ln, m * 128:(m + 1) * 128],
                                             ef[:ln],
                                             start=(k == 0), stop=(k == 4))
                    fm = dpool.tile([128, B, R * C], BF, tag="fm", name="fm",
                                    bufs=2)
                    mk = maskg[m].unsqueeze(1).broadcast_to([128, B, R * C])
                    nc.vector.tensor_mul(fm, F4[:, :, :R * C], mk)
                    gf = dpool.tile([128, B, C], F32, tag="gf", name="gf")
                    nc.vector.reduce_sum(
                        gf, fm.rearrange("p b (r c) -> p b c r", c=C), axis=AX.X)
                    nc.vector.tensor_copy(Gp[m], gf)
            return Gp

        def s_matmuls(spool, rhs_pair):
            s4T = spool.tile([16, B, C], F32, tag="s4T", name="s4T")
            for c in range(C):
                for m in range(2):
                    rhs = rhs_pair[m]
                    if len(rhs.shape) == 3:
                        rhs = rhs[:, :, c]
                    nc.tensor.matmul(s4T[:, :, c],
                                     ws_t[m][:, c * 16:(c + 1) * 16],
                                     rhs, start=(m == 0), stop=(m == 1))
            return s4T

        # ---- iter 0: uniform coupling ----
        capsum_bf = [rpool.tile([128, B], BF, tag=f"csb{g}", name=f"csb{g}") for g in range(2)]
        for g in range(2):
            nc.vector.tensor_scalar_mul(capsum_bf[g], capsum[g], 1.0 / C)
        with tc.tile_pool(name="sps0", bufs=1, space="PSUM") as sps:
            s4T = s_matmuls(sps, capsum_bf)
            v4T = v_squash(s4T, last=False)
        b_update(v4T, it=0)

        # ---- iters 1, 2 ----
        for it in (1, 2):
            Gp = softmax_G()
            with tc.tile_pool(name=f"sps{it}", bufs=1, space="PSUM") as sps:
                s4T = s_matmuls(sps, Gp)
                v4T = v_squash(s4T, last=(it == 2))
            if it == 1:
                b_update(v4T, it=1)

    nc.compile()
    return nc


@functools.lru_cache(maxsize=1)
def _get_nc():
    return _build_nc()


def _prep_consts(conv1_w, conv1_b, conv2_w, conv2_b, route_w):
    bf = ml_dtypes.bfloat16
    f32 = np.float32
    w1 = np.zeros((256, 256), f32)
    w1[:K1] = conv1_w.astype(f32).transpose(1, 2, 3, 0).reshape(K1, 256)
    w2 = (conv2_w.astype(f32)
          .reshape(2, 128, 2, 128, 81)       # [og, mo, ig, ki, tap]
          .transpose(2, 0, 3, 4, 1))         # [ig, og, ki, tap, mo] (contiguous DMA)
    ws = route_w.astype(f32).transpose(0, 2, 1, 3).reshape(256, C * O)
    wcob = route_w.astype(f32).transpose(3, 1, 0, 2).reshape(O, C, 256)
    maskg = np.zeros((2, 128, R * C), f32)
    for m in range(2):
        for j in range(128):
            r = m * 16 + j // D
            maskg[m, j, r * C:(r + 1) * C] = 1.0
    return {
        "w1": np.ascontiguousarray(w1).astype(bf),
        "b1": np.ascontiguousarray(conv1_b.astype(f32).reshape(256, 1)),
        "w2": np.ascontiguousarray(w2).reshape(2, 2, 128, 81 * 128).astype(bf),
        "b2": np.ascontiguousarray(conv2_b.astype(f32).reshape(256, 1)),
        "ws": np.ascontiguousarray(ws).astype(bf),
        "wcob": np.ascontiguousarray(wcob).astype(bf),
        "idf": np.eye(128, dtype=f32),
        "idb": np.eye(128, dtype=f32).astype(bf),
        "maskg": maskg,
    }


def _ensure_ntff_hook():
    """The agent image's antenv lacks axon_hooks; shim it so trace=True works."""
    import sys
    import types
    try:
        from antenv import axon_hooks  # noqa: F401
        return
    except ImportError:
        pass
    mod = types.ModuleType("antenv.axon_hooks")
    _h = [None]
    mod.get_axon_ntff_profile_hook = lambda: _h[0]
    mod.set_axon_ntff_profile_hook = lambda h: _h.__setitem__(0, h)
    sys.modules["antenv.axon_hooks"] = mod
    try:
        from trn_agent_boot.trn_boot import _ntff_profile_via_ctypes
        mod.set_axon_ntff_profile_hook(
            _ntff_profile_via_ctypes("/opt/axon/libaxon_pjrt.so"))
    except Exception as e:  # degrade: trace skipped, run still works
        print(f"ntff hook shim failed: {e}")


def run(x, conv1_w, conv1_b, conv2_w, conv2_b, route_w, trace=False, cores=NCORES):
    if trace:
        _ensure_ntff_hook()
    x = np.asarray(x, np.float32)
    nb = x.shape[0]
    consts = _prep_consts(np.asarray(conv1_w), np.asarray(conv1_b),
                          np.asarray(conv2_w), np.asarray(conv2_b),
                          np.asarray(route_w))
    win = np.lib.stride_tricks.sliding_window_view(x, (9, 9), axis=(2, 3))
    xb = (win.transpose(0, 1, 4, 5, 2, 3)          # [b, c, kh, kw, y, x]
          .reshape(nb, K1, NPIX1).astype(ml_dtypes.bfloat16))
    assert nb == B * cores
    in_maps = []
    for cid in range(cores):
        m = dict(consts)
        m["x"] = np.ascontiguousarray(xb[cid * B:(cid + 1) * B])
        in_maps.append(m)
    res = run_bass_kernel_spmd(_get_nc(), in_maps, list(range(cores)), trace=trace)
    out = np.concatenate([r["v_out"].reshape(B, C, O) for r in res.results], axis=0)
    return out.astype(np.float32), res


def kernel(x, conv1_w, conv1_b, conv2_w, conv2_b, route_w):
    out, _ = run(x, conv1_w, conv1_b, conv2_w, conv2_b, route_w, trace=False)
    return out

